# revision 50
# baseline (speedup 1.0000x reference)
"""Trainium2 Bass kernel for nn_AttentionHead (B=4, S=4096, D=512).

reference:
    K = x @ Wk.T; Q = x @ Wq.T; V = x @ Wv.T            # [B,S,D]
    scores[b,s,t] = <K[b,s], Q[b,t]> / sqrt(D)
    scores[b,:,t] = -1e12 where mask[b,t]==0
    out = softmax(scores, axis=t) @ V                    # [B,S,D]

Sharding: 8 cores = 4 batches x 2 sequence halves (rows s of the score
matrix). No collectives; each core computes Q^T/V for the full compacted
sequence of its batch and K^T for its s-half only. (A pairwise AllGather
that deduplicates the Q/V projections measured 230us SLOWER at ~40GB/s
effective inter-core bandwidth.)

Optimizations (measured on HW, cumulative 354.5us -> 197.7us):

1. Mask compaction (host-side, pure gather): masked key positions t get
   weight exactly 0 after softmax (exp(-1e12*scale) == 0 in fp32), so
   their Q/V columns and score columns are dead work. The host gathers
   the ~50% surviving t columns of x^T (zero-padded to a multiple of
   128) and all t loops run over the compacted width SKP. Padded slots
   get bias -1e9 inside the EXP so they contribute exactly 0.

2. bf16 everywhere on the PE: same 1 column/cycle rate as f32r, but the
   weight (stationary) loads use FWL (2x bandwidth, f32r gets none), so
   back-to-back matmuls run at the 518-cycle floor (216ns vs 230ns f32r,
   HW-measured). fp32->bf16 casts ride idle DVE cycles in phase 1;
   K^T/Q^T/V/P^T are quantized for free by the ACT-engine PSUM->SBUF
   copies / EXP. fp8 DoubleRow was tried and measured: score-side e4m3
   fails the 2e-2 gate outright (7e-2+, softmax argmax flips), and even
   AV-only e4m3 P/V gives 5e-2 on concentrated-softmax rows. Walrus
   also forbids mixing 32-bit and 16-bit matmul inputs, so bf16 applies
   to whole matmuls. Measured end-to-end err: 7.0e-3 (gate 2e-2).

3. Dataflow: K^T DMA+compute first so phase 2 can start earliest; x
   chunk DMAs ride only the sync/gpsimd queues (the scalar queue stalls
   DMA issues behind dependent ACT copies); each s-chunk's softmax
   epilogue (den/broadcast matmuls + reciprocal chain) is DELAYED into
   the next chunk's score stream so the PE never waits on the DVE
   reciprocal; final scale reads the broadcast PSUM directly.

Masking: mbias[t] = (mask[t]-1)*1e9 added inside the EXP; masked/padded
keys underflow to exactly 0 -- identical to the reference's -1e12 fill
followed by softmax (requires >=1 unmasked key per batch, which random
0/1 masks over 4096 positions guarantee).

Host passes x^T / W^T layouts and the t-gather (pure permutations /
selection; all FLOPs stay on device). The f32r DRAM declaration lets
raw fp32 bits feed the on-device bf16 casts directly.
"""

import numpy as np

import concourse.bacc as bacc
import concourse.mybir as mybir
from concourse.bass_utils import run_bass_kernel_spmd
from concourse.tile import TileContext

B, S, D = 4, 4096, 512
SH = S // 2          # per-core s rows (half sequence)
P = 128              # partition tile
CH = 512             # free-dim chunk
KD = D // P          # 4 contraction tiles over d
SCALE = 1.0 / float(np.sqrt(D))

F32 = mybir.dt.float32
F32R = mybir.dt.float32r
BF16 = mybir.dt.bfloat16
COPY = mybir.ActivationFunctionType.Copy
EXP = mybir.ActivationFunctionType.Exp

_CACHE = {}


def _build(skp):
    ntk = skp // P       # t-tiles over compacted keys

    nc = bacc.Bacc(num_devices=8)
    xsT = nc.declare_dram_parameter("xsT", [D, SH], F32R, isOutput=False)
    xkT = nc.declare_dram_parameter("xkT", [D, skp], F32R, isOutput=False)
    wqT = nc.declare_dram_parameter("wqT", [D, D], F32R, isOutput=False)
    wkT = nc.declare_dram_parameter("wkT", [D, D], F32R, isOutput=False)
    wvT = nc.declare_dram_parameter("wvT", [D, D], F32R, isOutput=False)
    maskT = nc.declare_dram_parameter("maskT", [P, ntk], F32, isOutput=False)
    outT = nc.declare_dram_parameter("outT", [D, SH], F32, isOutput=True)

    # Q/V-projection chunks over the compacted width (last may be short)
    qchunks = []
    c0 = 0
    while c0 < skp:
        w = min(CH, skp - c0)
        qchunks.append((c0, w))
        c0 += w

    with TileContext(nc) as tc:
        with tc.tile_pool(name="pers", bufs=1) as pers:
            qT = pers.tile([P, KD, skp], BF16)       # [d-par, d-tile, t]
            kT = pers.tile([P, KD, SH], BF16)        # [d-par, d-tile, s]
            vA = pers.tile([P, ntk, D], BF16)        # [t-par, t-tile, d]
            mk = pers.tile([P, ntk], F32)
            ones = pers.tile([1, P], F32R)
            ones32 = pers.tile([1, P], F32)
            onec = pers.tile([P, 1], F32R)           # den column-sum weights
            onec32 = pers.tile([P, 1], F32)
            mbias = pers.tile([P, ntk], F32)

            # ---------------- phase 1: projections ----------------
            with tc.tile_pool(name="stage", bufs=1) as stage, \
                 tc.tile_pool(name="ppsum", bufs=3, space="PSUM") as ppsum:
                wq32 = stage.tile([P, KD * D], F32R, tag="wq32")
                wk32 = stage.tile([P, KD * D], F32R, tag="wk32")
                wv32 = stage.tile([P, KD * D], F32R, tag="wv32")
                wq = stage.tile([P, KD * D], BF16, tag="wq")
                wk = stage.tile([P, KD * D], BF16, tag="wk")
                wv = stage.tile([P, KD * D], BF16, tag="wv")
                # PE warm-up: dummy matmuls into a trash PSUM bank while the
                # first DMAs are in flight -- keeps the HAM clock-gate at
                # 2.4GHz so the real matmuls start warm instead of paying
                # the ~3.4us half-rate ramp. (Variants measured: no warm-up
                # pushes the cold ramp onto the real K matmuls and loses
                # ~1us; a DMA-fed warm-up starting earlier measures the
                # same within noise -- phase 1 is HBM-bound either way.)
                warm = stage.tile([P, CH], BF16, tag="warm")
                nc.vector.memset(warm, 0.0)
                for r in range(24):
                    wps = ppsum.tile([P, CH], F32, tag="warm", bufs=2,
                                     name="wps")
                    nc.tensor.matmul(wps, warm[:, 0:P], warm,
                                     start=True, stop=True)

                dmae = [nc.sync, nc.gpsimd]
                # K-path loads FIRST (wk + all xsT chunks): phase 1 start-up
                # is HBM-bound, so everything the K pipeline needs outranks
                # wq/wv/xk. Pair each k-tile's (wk, x) on one queue so the
                # first K matmul waits on a single queue-sem.
                xrk32 = []
                for c in range(SH // CH):
                    xrk32.append(stage.tile([P, KD * CH], F32R, tag="xr32",
                                            bufs=4, name=f"xrk32_{c}"))
                for j in range(KD):
                    eng = dmae[j % 2]
                    eng.dma_start(
                        out=wk32[:, j * D:(j + 1) * D],
                        in_=wkT[j * P:(j + 1) * P, :])
                    eng.dma_start(
                        out=xrk32[0][:, j * CH:(j + 1) * CH],
                        in_=xsT[j * P:(j + 1) * P, 0:CH])
                for j in range(KD):
                    nc.vector.tensor_copy(out=wk[:, j * D:(j + 1) * D],
                                          in_=wk32[:, j * D:(j + 1) * D])
                for c in range(1, SH // CH):
                    for j in range(KD):
                        dmae[(c + j) % 2].dma_start(
                            out=xrk32[c][:, j * CH:(j + 1) * CH],
                            in_=xsT[j * P:(j + 1) * P, c * CH:(c + 1) * CH])
                for j in range(KD):
                    nc.sync.dma_start(out=wq32[:, j * D:(j + 1) * D],
                                      in_=wqT[j * P:(j + 1) * P, :])
                    nc.gpsimd.dma_start(out=wv32[:, j * D:(j + 1) * D],
                                        in_=wvT[j * P:(j + 1) * P, :])

                # constants + mask bias (off the first-wave critical path)
                nc.scalar.dma_start(out=mk, in_=maskT[:, :])
                nc.vector.memset(ones32, 1.0)
                nc.vector.tensor_copy(out=ones, in_=ones32)
                nc.vector.memset(onec32, 1.0)
                nc.vector.tensor_copy(out=onec, in_=onec32)
                # mbias[p, i] = (mask-1)*1e9: 0 where kept, -1e9 where
                # masked/padded; exp(score*scale + mbias) underflows to 0
                nc.vector.tensor_scalar(mbias, mk, -1.0, 1.0e9,
                                        mybir.AluOpType.add,
                                        mybir.AluOpType.mult)

                def cast_chunk(xr32, w):
                    """fp32 -> bf16 x-chunk cast, per k-tile slice so the
                    first matmul only waits for its own slice."""
                    xr = stage.tile([P, KD * CH], BF16, tag="xr", bufs=3,
                                    name="xr")
                    for j in range(KD):
                        nc.vector.tensor_copy(
                            out=xr[:, j * CH:j * CH + w],
                            in_=xr32[:, j * CH:j * CH + w])
                    return xr

                # K^T first (phase 2's first score groups need it earliest).
                # Chunk 0 runs f32r STRAIGHT from the DMA'd tiles: no DVE
                # cast in its dependency chain, so the PE starts ~7us
                # earlier (right behind the first DMA wave), and these real
                # matmuls double as the HAM clock warm-up -- no synthetic
                # warm-up needed.
                for c in range(SH // CH):
                    xr = cast_chunk(xrk32[c], CH)
                    for jo in range(KD):
                        pq = ppsum.tile([P, CH], F32, tag="pq", name="pqk")
                        for kd in range(KD):
                            nc.tensor.matmul(
                                pq,
                                wk[:, kd * D + jo * P: kd * D + (jo + 1) * P],
                                xr[:, kd * CH:(kd + 1) * CH],
                                start=(kd == 0), stop=(kd == KD - 1))
                        nc.scalar.activation(
                            out=kT[:, jo, c * CH:(c + 1) * CH],
                            in_=pq, func=COPY)
                    if c == 0:
                        # wq/wv casts on GPSIMD (idle in phase 1, and these
                        # aren't needed until ~25us): keeping them off the
                        # DVE FIFO stops them delaying the x-chunk casts
                        # that gate the K/Q matmul stream
                        for j in range(KD):
                            nc.gpsimd.tensor_copy(
                                out=wq[:, j * D:(j + 1) * D],
                                in_=wq32[:, j * D:(j + 1) * D])
                        nc.gpsimd.tensor_copy(out=wv, in_=wv32)

                # Q^T and V from the compacted x^T, chunk by chunk
                for ci, (c0, w) in enumerate(qchunks):
                    xr32 = stage.tile([P, KD * CH], F32R, tag="xr32", bufs=4,
                                      name="xrq32")
                    for j in range(KD):
                        dmae[(ci + j) % 2].dma_start(
                            out=xr32[:, j * CH:j * CH + w],
                            in_=xkT[j * P:(j + 1) * P, c0:c0 + w])
                    xr = cast_chunk(xr32, w)
                    for jo in range(KD):
                        pq = ppsum.tile([P, CH], F32, tag="pq")
                        for kd in range(KD):
                            nc.tensor.matmul(
                                pq[:, 0:w],
                                wq[:, kd * D + jo * P: kd * D + (jo + 1) * P],
                                xr[:, kd * CH:kd * CH + w],
                                start=(kd == 0), stop=(kd == KD - 1))
                        nc.scalar.activation(
                            out=qT[:, jo, c0:c0 + w],
                            in_=pq[:, 0:w], func=COPY)
                    for tt in range(w // P):
                        ti = c0 // P + tt
                        pv = ppsum.tile([P, D], F32, tag="pv")
                        for kd in range(KD):
                            nc.tensor.matmul(
                                pv,
                                xr[:, kd * CH + tt * P: kd * CH + (tt + 1) * P],
                                wv[:, kd * D:(kd + 1) * D],
                                start=(kd == 0), stop=(kd == KD - 1))
                        nc.scalar.activation(
                            out=vA[:, ti, :], in_=pv, func=COPY)

            # ---------------- phase 2: attention ----------------
            with tc.tile_pool(name="att", bufs=1) as att, \
                 tc.tile_pool(name="apsum", bufs=1, space="PSUM") as apsum:

                nchunk = SH // CH
                pending = [None]     # delayed epilogue from previous chunk

                def make_epilogue(den128, osb, sc):
                    def emit():
                        # den[s] = column sum of den128 (P^T already masked
                        # by the EXP bias), then out *= 1/den via a rank-1
                        # broadcast matmul; the scale mult reads the
                        # broadcast PSUM directly (no SBUF staging copy)
                        dps = apsum.tile([1, CH], F32, tag="bc", name="dps")
                        nc.tensor.matmul(dps, onec, den128,
                                         start=True, stop=True)
                        rec = att.tile([1, CH], F32, tag="rec")
                        nc.vector.reciprocal_approx_fast(out=rec, in_=dps)
                        recr = att.tile([1, CH], F32R, tag="recr")
                        nc.vector.tensor_copy(out=recr, in_=rec)
                        bps = apsum.tile([P, CH], F32, tag="bc", name="bps")
                        nc.tensor.matmul(bps, ones, recr,
                                         start=True, stop=True)
                        dma_engs = [nc.sync, nc.gpsimd, nc.scalar, nc.sync]
                        for d in range(KD):
                            fin = att.tile([P, CH], F32, tag=f"fin{d % 2}",
                                           name=f"fin{d}", bufs=2)
                            nc.vector.tensor_mul(fin, osb[d], bps)
                            dma_engs[d].dma_start(
                                out=outT[d * P:(d + 1) * P,
                                         sc * CH:(sc + 1) * CH],
                                in_=fin)
                    return emit

                for sc in range(nchunk):
                    opsum = [apsum.tile([P, CH], F32, tag=f"o{d}",
                                        name=f"opsum{d}")
                             for d in range(KD)]
                    # den128 accumulates P^T on the DVE (off the PE); bufs=2
                    # because the delayed den matmul still reads chunk sc's
                    # accumulator while chunk sc+1 starts a fresh one
                    den128 = att.tile([P, CH], F32R, tag="den128", bufs=2)

                    def s_group(ti, sc=sc):
                        ss = apsum.tile([P, CH], F32, tag="s", bufs=3)
                        for kd in range(KD):
                            nc.tensor.matmul(
                                ss,
                                qT[:, kd, ti * P:(ti + 1) * P],
                                kT[:, kd, sc * CH:(sc + 1) * CH],
                                start=(kd == 0), stop=(kd == KD - 1))
                        return ss

                    ss_cur = s_group(0)
                    for ti in range(ntk):
                        ss_next = s_group(ti + 1) if ti + 1 < ntk else None
                        pt = att.tile([P, CH], BF16, tag="pt", bufs=3)
                        # masked softmax numerator: exp(score*scale + mbias)
                        nc.scalar.activation(out=pt, in_=ss_cur, func=EXP,
                                             scale=SCALE,
                                             bias=mbias[:, ti:ti + 1])
                        for d in range(KD):
                            nc.tensor.matmul(
                                opsum[d],
                                vA[:, ti, d * P:(d + 1) * P],
                                pt, start=(ti == 0), stop=(ti == ntk - 1))
                        if ti == 0:
                            nc.vector.tensor_copy(out=den128, in_=pt)
                        else:
                            nc.vector.tensor_add(den128, den128, pt)
                        if ti == 2 and pending[0] is not None:
                            # previous chunk's epilogue: its den/broadcast
                            # matmuls slot in here so the PE never idles
                            # waiting on the DVE reciprocal chain
                            pending[0]()
                            pending[0] = None
                        ss_cur = ss_next

                    if sc < nchunk - 1:
                        # drain the AV accumulators now (frees the PSUM
                        # banks for the next chunk); the rest of the
                        # epilogue waits for the next chunk's stream
                        osb = []
                        for d in range(KD):
                            ot = att.tile([P, CH], F32, tag=f"osb{d}",
                                          name=f"osb{d}")
                            nc.vector.tensor_copy(out=ot, in_=opsum[d])
                            osb.append(ot)
                        pending[0] = make_epilogue(den128, osb, sc)
                    else:
                        # last chunk: run the reciprocal chain FIRST (the
                        # drains would otherwise queue ahead of it on the
                        # DVE FIFO and stretch the exposed tail), then
                        # interleave drain(d) -> scale(d) -> store(d) so
                        # each output block ships as soon as it's ready
                        dps = apsum.tile([1, CH], F32, tag="bc", name="dps")
                        nc.tensor.matmul(dps, onec, den128,
                                         start=True, stop=True)
                        rec = att.tile([1, CH], F32, tag="rec")
                        nc.vector.reciprocal_approx_fast(out=rec, in_=dps)
                        recr = att.tile([1, CH], F32R, tag="recr")
                        nc.vector.tensor_copy(out=recr, in_=rec)
                        bps = apsum.tile([P, CH], F32, tag="bc", name="bps")
                        nc.tensor.matmul(bps, ones, recr,
                                         start=True, stop=True)
                        # drains ride the ACT engine (idle after the last
                        # EXP): the DVE otherwise serializes ~6us of
                        # recip+drains+mults behind the final AV matmul
                        dma_engs = [nc.sync, nc.gpsimd, nc.scalar, nc.sync]
                        for d in range(KD):
                            ot = att.tile([P, CH], F32, tag=f"osb{d}",
                                          name=f"osb{d}")
                            nc.scalar.activation(out=ot, in_=opsum[d],
                                                 func=COPY)
                            fin = att.tile([P, CH], F32, tag=f"fin{d % 2}",
                                           name=f"fin{d}", bufs=2)
                            nc.vector.tensor_mul(fin, ot, bps)
                            dma_engs[d].dma_start(
                                out=outT[d * P:(d + 1) * P,
                                         sc * CH:(sc + 1) * CH],
                                in_=fin)

    nc.compile()
    return nc


def _prep(x, mask, Wk, Wq, Wv):
    """Host-side layout prep: transposes + mask-compaction gather.
    Returns (skp, in_maps)."""
    x = np.asarray(x, dtype=np.float32)
    mask_np = np.asarray(mask)
    wqT = np.ascontiguousarray(np.asarray(Wq, dtype=np.float32).T)
    wkT = np.ascontiguousarray(np.asarray(Wk, dtype=np.float32).T)
    wvT = np.ascontiguousarray(np.asarray(Wv, dtype=np.float32).T)

    idxs = [np.nonzero(mask_np[b])[0] for b in range(B)]
    nk_max = max(len(ix) for ix in idxs)
    skp = max(256, ((nk_max + 127) // 128) * 128)
    ntk = skp // P

    in_maps = []
    for b in range(B):
        xT = np.ascontiguousarray(x[b].T)                  # [D, S]
        xk = np.zeros((D, skp), dtype=np.float32)
        xk[:, :len(idxs[b])] = xT[:, idxs[b]]
        mg = np.zeros(skp, dtype=np.float32)
        mg[:len(idxs[b])] = 1.0
        mkT = np.ascontiguousarray(mg.reshape(ntk, P).T)   # [P, ntk]
        for h in range(2):
            in_maps.append({
                "xsT": np.ascontiguousarray(xT[:, h * SH:(h + 1) * SH]),
                "xkT": xk,
                "wqT": wqT, "wkT": wkT, "wvT": wvT,
                "maskT": mkT,
            })
    return skp, in_maps


def _get_nc(skp):
    if skp not in _CACHE:
        _CACHE[skp] = _build(skp)
    return _CACHE[skp]


def kernel(x, mask, Wk, Wq, Wv):
    skp, in_maps = _prep(x, mask, Wk, Wq, Wv)
    nc = _get_nc(skp)

    res = run_bass_kernel_spmd(nc, in_maps, core_ids=list(range(8)))

    out = np.empty((B, S, D), dtype=np.float32)
    for b in range(B):
        for h in range(2):
            out[b, h * SH:(h + 1) * SH, :] = res.results[2 * b + h]["outT"].T
    return out


# revision 51
# speedup vs baseline: 1.0563x; 1.0563x over previous
"""Trainium2 Bass kernel for nn_AttentionHead (B=4, S=4096, D=512).

reference:
    K = x @ Wk.T; Q = x @ Wq.T; V = x @ Wv.T            # [B,S,D]
    scores[b,s,t] = <K[b,s], Q[b,t]> / sqrt(D)
    scores[b,:,t] = -1e12 where mask[b,t]==0
    out = softmax(scores, axis=t) @ V                    # [B,S,D]

Sharding: 8 cores = 4 batches x 2 sequence halves (rows s of the score
matrix). No collectives; each core computes Q^T/V for the full compacted
sequence of its batch and K^T for its s-half only. (A pairwise AllGather
that deduplicates the Q/V projections measured 230us SLOWER at ~40GB/s
effective inter-core bandwidth.)

Optimizations (measured on HW, cumulative 354.5us -> 197.7us):

1. Mask compaction (host-side, pure gather): masked key positions t get
   weight exactly 0 after softmax (exp(-1e12*scale) == 0 in fp32), so
   their Q/V columns and score columns are dead work. The host gathers
   the ~50% surviving t columns of x^T (zero-padded to a multiple of
   128) and all t loops run over the compacted width SKP. Padded slots
   get bias -1e9 inside the EXP so they contribute exactly 0.

2. bf16 everywhere on the PE: same 1 column/cycle rate as f32r, but the
   weight (stationary) loads use FWL (2x bandwidth, f32r gets none), so
   back-to-back matmuls run at the 518-cycle floor (216ns vs 230ns f32r,
   HW-measured). fp32->bf16 casts ride idle DVE cycles in phase 1;
   K^T/Q^T/V/P^T are quantized for free by the ACT-engine PSUM->SBUF
   copies / EXP. fp8 DoubleRow was tried and measured: score-side e4m3
   fails the 2e-2 gate outright (7e-2+, softmax argmax flips), and even
   AV-only e4m3 P/V gives 5e-2 on concentrated-softmax rows. Walrus
   also forbids mixing 32-bit and 16-bit matmul inputs, so bf16 applies
   to whole matmuls. Measured end-to-end err: 7.0e-3 (gate 2e-2).

3. Dataflow: K^T DMA+compute first so phase 2 can start earliest; x
   chunk DMAs ride only the sync/gpsimd queues (the scalar queue stalls
   DMA issues behind dependent ACT copies); each s-chunk's softmax
   epilogue (den/broadcast matmuls + reciprocal chain) is DELAYED into
   the next chunk's score stream so the PE never waits on the DVE
   reciprocal; final scale reads the broadcast PSUM directly.

Masking: mbias[t] = (mask[t]-1)*1e9 added inside the EXP; masked/padded
keys underflow to exactly 0 -- identical to the reference's -1e12 fill
followed by softmax (requires >=1 unmasked key per batch, which random
0/1 masks over 4096 positions guarantee).

Host passes x^T / W^T layouts and the t-gather (pure permutations /
selection; all FLOPs stay on device). The f32r DRAM declaration lets
raw fp32 bits feed the on-device bf16 casts directly.
"""

import numpy as np

import concourse.bacc as bacc
import concourse.mybir as mybir
from concourse.bass_utils import run_bass_kernel_spmd
from concourse.tile import TileContext

B, S, D = 4, 4096, 512
SH = S // 2          # per-core s rows (half sequence)
P = 128              # partition tile
CH = 512             # free-dim chunk
KD = D // P          # 4 contraction tiles over d
SCALE = 1.0 / float(np.sqrt(D))

F32 = mybir.dt.float32
F32R = mybir.dt.float32r
BF16 = mybir.dt.bfloat16
COPY = mybir.ActivationFunctionType.Copy
EXP = mybir.ActivationFunctionType.Exp

_CACHE = {}


def _build(skp):
    ntk = skp // P       # t-tiles over compacted keys

    nc = bacc.Bacc(num_devices=8)
    xsT = nc.declare_dram_parameter("xsT", [D, SH], F32R, isOutput=False)
    xkT = nc.declare_dram_parameter("xkT", [D, skp], F32R, isOutput=False)
    wqT = nc.declare_dram_parameter("wqT", [D, D], F32R, isOutput=False)
    wkT = nc.declare_dram_parameter("wkT", [D, D], F32R, isOutput=False)
    wvT = nc.declare_dram_parameter("wvT", [D, D], F32R, isOutput=False)
    maskT = nc.declare_dram_parameter("maskT", [P, ntk], F32, isOutput=False)
    outT = nc.declare_dram_parameter("outT", [D, SH], F32, isOutput=True)

    # Q/V-projection chunks over the compacted width (last may be short)
    qchunks = []
    c0 = 0
    while c0 < skp:
        w = min(CH, skp - c0)
        qchunks.append((c0, w))
        c0 += w

    with TileContext(nc) as tc:
        with tc.tile_pool(name="pers", bufs=1) as pers:
            qT = pers.tile([P, KD, skp], BF16)       # [d-par, d-tile, t]
            kT = pers.tile([P, KD, SH], BF16)        # [d-par, d-tile, s]
            vA = pers.tile([P, ntk, D], BF16)        # [t-par, t-tile, d]
            mk = pers.tile([P, ntk], F32)
            ones = pers.tile([1, P], F32R)
            ones32 = pers.tile([1, P], F32)
            onec = pers.tile([P, 1], F32R)           # den column-sum weights
            onec32 = pers.tile([P, 1], F32)
            mbias = pers.tile([P, ntk], F32)

            # ---------------- phase 1: projections ----------------
            with tc.tile_pool(name="stage", bufs=1) as stage, \
                 tc.tile_pool(name="ppsum", bufs=3, space="PSUM") as ppsum:
                wq32 = stage.tile([P, KD * D], F32R, tag="wq32")
                wk32 = stage.tile([P, KD * D], F32R, tag="wk32")
                wv32 = stage.tile([P, KD * D], F32R, tag="wv32")
                wq = stage.tile([P, KD * D], BF16, tag="wq")
                wk = stage.tile([P, KD * D], BF16, tag="wk")
                wv = stage.tile([P, KD * D], BF16, tag="wv")
                # PE warm-up: dummy matmuls into a trash PSUM bank while the
                # first DMAs are in flight -- keeps the HAM clock-gate at
                # 2.4GHz so the real matmuls start warm instead of paying
                # the ~3.4us half-rate ramp. (Variants measured: no warm-up
                # pushes the cold ramp onto the real K matmuls and loses
                # ~1us; a DMA-fed warm-up starting earlier measures the
                # same within noise -- phase 1 is HBM-bound either way.)
                warm = stage.tile([P, CH], BF16, tag="warm")
                nc.vector.memset(warm, 0.0)
                for r in range(24):
                    wps = ppsum.tile([P, CH], F32, tag="warm", bufs=2,
                                     name="wps")
                    nc.tensor.matmul(wps, warm[:, 0:P], warm,
                                     start=True, stop=True)

                dmae = [nc.sync, nc.gpsimd]
                # K-path loads FIRST (wk + all xsT chunks): phase 1 start-up
                # is HBM-bound, so everything the K pipeline needs outranks
                # wq/wv/xk. Pair each k-tile's (wk, x) on one queue so the
                # first K matmul waits on a single queue-sem.
                xrk32 = []
                for c in range(SH // CH):
                    xrk32.append(stage.tile([P, KD * CH], F32R, tag="xr32",
                                            bufs=4, name=f"xrk32_{c}"))
                for j in range(KD):
                    eng = dmae[j % 2]
                    eng.dma_start(
                        out=wk32[:, j * D:(j + 1) * D],
                        in_=wkT[j * P:(j + 1) * P, :])
                    eng.dma_start(
                        out=xrk32[0][:, j * CH:(j + 1) * CH],
                        in_=xsT[j * P:(j + 1) * P, 0:CH])
                for j in range(KD):
                    nc.vector.tensor_copy(out=wk[:, j * D:(j + 1) * D],
                                          in_=wk32[:, j * D:(j + 1) * D])
                for c in range(1, SH // CH):
                    for j in range(KD):
                        dmae[(c + j) % 2].dma_start(
                            out=xrk32[c][:, j * CH:(j + 1) * CH],
                            in_=xsT[j * P:(j + 1) * P, c * CH:(c + 1) * CH])
                for j in range(KD):
                    nc.sync.dma_start(out=wq32[:, j * D:(j + 1) * D],
                                      in_=wqT[j * P:(j + 1) * P, :])
                    nc.gpsimd.dma_start(out=wv32[:, j * D:(j + 1) * D],
                                        in_=wvT[j * P:(j + 1) * P, :])

                # constants + mask bias (off the first-wave critical path)
                nc.scalar.dma_start(out=mk, in_=maskT[:, :])
                nc.vector.memset(ones32, 1.0)
                nc.vector.tensor_copy(out=ones, in_=ones32)
                nc.vector.memset(onec32, 1.0)
                nc.vector.tensor_copy(out=onec, in_=onec32)
                # mbias[p, i] = (mask-1)*1e9: 0 where kept, -1e9 where
                # masked/padded; exp(score*scale + mbias) underflows to 0
                nc.vector.tensor_scalar(mbias, mk, -1.0, 1.0e9,
                                        mybir.AluOpType.add,
                                        mybir.AluOpType.mult)

                def cast_chunk(xr32, w):
                    """fp32 -> bf16 x-chunk cast, per k-tile slice so the
                    first matmul only waits for its own slice."""
                    xr = stage.tile([P, KD * CH], BF16, tag="xr", bufs=3,
                                    name="xr")
                    for j in range(KD):
                        nc.vector.tensor_copy(
                            out=xr[:, j * CH:j * CH + w],
                            in_=xr32[:, j * CH:j * CH + w])
                    return xr

                # K^T first (phase 2's first score groups need it earliest).
                # Chunk 0 runs f32r STRAIGHT from the DMA'd tiles: no DVE
                # cast in its dependency chain, so the PE starts ~7us
                # earlier (right behind the first DMA wave), and these real
                # matmuls double as the HAM clock warm-up -- no synthetic
                # warm-up needed.
                for c in range(SH // CH):
                    xr = cast_chunk(xrk32[c], CH)
                    for jo in range(KD):
                        pq = ppsum.tile([P, CH], F32, tag="pq", name="pqk")
                        for kd in range(KD):
                            nc.tensor.matmul(
                                pq,
                                wk[:, kd * D + jo * P: kd * D + (jo + 1) * P],
                                xr[:, kd * CH:(kd + 1) * CH],
                                start=(kd == 0), stop=(kd == KD - 1))
                        nc.scalar.activation(
                            out=kT[:, jo, c * CH:(c + 1) * CH],
                            in_=pq, func=COPY)
                    if c == 0:
                        for j in range(KD):
                            nc.vector.tensor_copy(
                                out=wq[:, j * D:(j + 1) * D],
                                in_=wq32[:, j * D:(j + 1) * D])
                        nc.vector.tensor_copy(out=wv, in_=wv32)

                # Q^T and V from the compacted x^T, chunk by chunk
                for ci, (c0, w) in enumerate(qchunks):
                    xr32 = stage.tile([P, KD * CH], F32R, tag="xr32", bufs=4,
                                      name="xrq32")
                    for j in range(KD):
                        dmae[(ci + j) % 2].dma_start(
                            out=xr32[:, j * CH:j * CH + w],
                            in_=xkT[j * P:(j + 1) * P, c0:c0 + w])
                    xr = cast_chunk(xr32, w)
                    for jo in range(KD):
                        pq = ppsum.tile([P, CH], F32, tag="pq")
                        for kd in range(KD):
                            nc.tensor.matmul(
                                pq[:, 0:w],
                                wq[:, kd * D + jo * P: kd * D + (jo + 1) * P],
                                xr[:, kd * CH:kd * CH + w],
                                start=(kd == 0), stop=(kd == KD - 1))
                        nc.scalar.activation(
                            out=qT[:, jo, c0:c0 + w],
                            in_=pq[:, 0:w], func=COPY)
                    for tt in range(w // P):
                        ti = c0 // P + tt
                        pv = ppsum.tile([P, D], F32, tag="pv")
                        for kd in range(KD):
                            nc.tensor.matmul(
                                pv,
                                xr[:, kd * CH + tt * P: kd * CH + (tt + 1) * P],
                                wv[:, kd * D:(kd + 1) * D],
                                start=(kd == 0), stop=(kd == KD - 1))
                        nc.scalar.activation(
                            out=vA[:, ti, :], in_=pv, func=COPY)

            # ---------------- phase 2: attention ----------------
            with tc.tile_pool(name="att", bufs=1) as att, \
                 tc.tile_pool(name="apsum", bufs=1, space="PSUM") as apsum:

                nchunk = SH // CH
                pending = [None]     # delayed epilogue from previous chunk

                def make_epilogue(den128, osb, sc):
                    def emit():
                        # den[s] = column sum of den128 (P^T already masked
                        # by the EXP bias), then out *= 1/den via a rank-1
                        # broadcast matmul; the scale mult reads the
                        # broadcast PSUM directly (no SBUF staging copy)
                        dps = apsum.tile([1, CH], F32, tag="bc", name="dps")
                        nc.tensor.matmul(dps, onec, den128,
                                         start=True, stop=True)
                        rec = att.tile([1, CH], F32, tag="rec")
                        nc.vector.reciprocal_approx_fast(out=rec, in_=dps)
                        recr = att.tile([1, CH], F32R, tag="recr")
                        nc.vector.tensor_copy(out=recr, in_=rec)
                        bps = apsum.tile([P, CH], F32, tag="bc", name="bps")
                        nc.tensor.matmul(bps, ones, recr,
                                         start=True, stop=True)
                        dma_engs = [nc.sync, nc.gpsimd, nc.scalar, nc.sync]
                        for d in range(KD):
                            fin = att.tile([P, CH], F32, tag=f"fin{d % 2}",
                                           name=f"fin{d}", bufs=2)
                            nc.vector.tensor_mul(fin, osb[d], bps)
                            dma_engs[d].dma_start(
                                out=outT[d * P:(d + 1) * P,
                                         sc * CH:(sc + 1) * CH],
                                in_=fin)
                    return emit

                for sc in range(nchunk):
                    opsum = [apsum.tile([P, CH], F32, tag=f"o{d}",
                                        name=f"opsum{d}")
                             for d in range(KD)]
                    # den128 accumulates P^T on the DVE (off the PE); bufs=2
                    # because the delayed den matmul still reads chunk sc's
                    # accumulator while chunk sc+1 starts a fresh one
                    den128 = att.tile([P, CH], F32R, tag="den128", bufs=2)

                    def s_group(ti, sc=sc):
                        ss = apsum.tile([P, CH], F32, tag="s", bufs=3)
                        for kd in range(KD):
                            nc.tensor.matmul(
                                ss,
                                qT[:, kd, ti * P:(ti + 1) * P],
                                kT[:, kd, sc * CH:(sc + 1) * CH],
                                start=(kd == 0), stop=(kd == KD - 1))
                        return ss

                    ss_cur = s_group(0)
                    for ti in range(ntk):
                        ss_next = s_group(ti + 1) if ti + 1 < ntk else None
                        pt = att.tile([P, CH], BF16, tag="pt", bufs=3)
                        # masked softmax numerator: exp(score*scale + mbias)
                        nc.scalar.activation(out=pt, in_=ss_cur, func=EXP,
                                             scale=SCALE,
                                             bias=mbias[:, ti:ti + 1])
                        for d in range(KD):
                            nc.tensor.matmul(
                                opsum[d],
                                vA[:, ti, d * P:(d + 1) * P],
                                pt, start=(ti == 0), stop=(ti == ntk - 1))
                        if ti == 0:
                            nc.vector.tensor_copy(out=den128, in_=pt)
                        else:
                            nc.vector.tensor_add(den128, den128, pt)
                        if ti == 2 and pending[0] is not None:
                            # previous chunk's epilogue: its den/broadcast
                            # matmuls slot in here so the PE never idles
                            # waiting on the DVE reciprocal chain
                            pending[0]()
                            pending[0] = None
                        ss_cur = ss_next

                    if sc < nchunk - 1:
                        # drain the AV accumulators now (frees the PSUM
                        # banks for the next chunk); the rest of the
                        # epilogue waits for the next chunk's stream
                        osb = []
                        for d in range(KD):
                            ot = att.tile([P, CH], F32, tag=f"osb{d}",
                                          name=f"osb{d}")
                            nc.vector.tensor_copy(out=ot, in_=opsum[d])
                            osb.append(ot)
                        pending[0] = make_epilogue(den128, osb, sc)
                    else:
                        # last chunk: run the reciprocal chain FIRST (the
                        # drains would otherwise queue ahead of it on the
                        # DVE FIFO and stretch the exposed tail), then
                        # interleave drain(d) -> scale(d) -> store(d) so
                        # each output block ships as soon as it's ready
                        dps = apsum.tile([1, CH], F32, tag="bc", name="dps")
                        nc.tensor.matmul(dps, onec, den128,
                                         start=True, stop=True)
                        rec = att.tile([1, CH], F32, tag="rec")
                        nc.vector.reciprocal_approx_fast(out=rec, in_=dps)
                        recr = att.tile([1, CH], F32R, tag="recr")
                        nc.vector.tensor_copy(out=recr, in_=rec)
                        bps = apsum.tile([P, CH], F32, tag="bc", name="bps")
                        nc.tensor.matmul(bps, ones, recr,
                                         start=True, stop=True)
                        # drains ride the ACT engine (idle after the last
                        # EXP): the DVE otherwise serializes ~6us of
                        # recip+drains+mults behind the final AV matmul
                        dma_engs = [nc.sync, nc.gpsimd, nc.scalar, nc.sync]
                        for d in range(KD):
                            ot = att.tile([P, CH], F32, tag=f"osb{d}",
                                          name=f"osb{d}")
                            nc.scalar.activation(out=ot, in_=opsum[d],
                                                 func=COPY)
                            fin = att.tile([P, CH], F32, tag=f"fin{d % 2}",
                                           name=f"fin{d}", bufs=2)
                            nc.vector.tensor_mul(fin, ot, bps)
                            dma_engs[d].dma_start(
                                out=outT[d * P:(d + 1) * P,
                                         sc * CH:(sc + 1) * CH],
                                in_=fin)

    nc.compile()
    return nc


def _prep(x, mask, Wk, Wq, Wv):
    """Host-side layout prep: transposes + mask-compaction gather.
    Returns (skp, in_maps)."""
    x = np.asarray(x, dtype=np.float32)
    mask_np = np.asarray(mask)
    wqT = np.ascontiguousarray(np.asarray(Wq, dtype=np.float32).T)
    wkT = np.ascontiguousarray(np.asarray(Wk, dtype=np.float32).T)
    wvT = np.ascontiguousarray(np.asarray(Wv, dtype=np.float32).T)

    idxs = [np.nonzero(mask_np[b])[0] for b in range(B)]
    nk_max = max(len(ix) for ix in idxs)
    skp = max(256, ((nk_max + 127) // 128) * 128)
    ntk = skp // P

    in_maps = []
    for b in range(B):
        xT = np.ascontiguousarray(x[b].T)                  # [D, S]
        xk = np.zeros((D, skp), dtype=np.float32)
        xk[:, :len(idxs[b])] = xT[:, idxs[b]]
        mg = np.zeros(skp, dtype=np.float32)
        mg[:len(idxs[b])] = 1.0
        mkT = np.ascontiguousarray(mg.reshape(ntk, P).T)   # [P, ntk]
        for h in range(2):
            in_maps.append({
                "xsT": np.ascontiguousarray(xT[:, h * SH:(h + 1) * SH]),
                "xkT": xk,
                "wqT": wqT, "wkT": wkT, "wvT": wvT,
                "maskT": mkT,
            })
    return skp, in_maps


def _get_nc(skp):
    if skp not in _CACHE:
        _CACHE[skp] = _build(skp)
    return _CACHE[skp]


def kernel(x, mask, Wk, Wq, Wv):
    skp, in_maps = _prep(x, mask, Wk, Wq, Wv)
    nc = _get_nc(skp)

    res = run_bass_kernel_spmd(nc, in_maps, core_ids=list(range(8)))

    out = np.empty((B, S, D), dtype=np.float32)
    for b in range(B):
        for h in range(2):
            out[b, h * SH:(h + 1) * SH, :] = res.results[2 * b + h]["outT"].T
    return out
